# revision 5
# baseline (speedup 1.0000x reference)
"""Trainium2 Bass kernel for AnyGPT local-attention block (8 NeuronCores).

Sharding: (batch, seq-half) -> 8 shards of 1024 query tokens each; every core
gets a 256-token k/v halo (zero-padded at sequence start), so no collectives
are needed and the host gather is a pure concatenation.

Per-core pipeline (all matmuls in bf16, LayerNorm/softmax math in fp32):
  qT/kT = W^T-major projections ([H, tok] layout), v natural ([tok, H]) with a
  built-in ones column for softmax denominators; banded scores computed
  TRANSPOSED ([key, query] layout) so exp'd probs feed the ctx matmul directly
  and ctx comes out pre-transposed for the output projection; softmax is
  unnormalized (no max subtraction; scores are O(30)) with the denominator
  recovered from the ones row and divided into ctx via a rank-1 broadcast.
"""

import numpy as np
import ml_dtypes

import concourse.bass as bass
import concourse.mybir as mybir
import concourse.tile as tile
from concourse import bacc

F32 = mybir.dt.float32
BF16 = mybir.dt.bfloat16

B, S, H, NH, HD, WIN = 4, 2048, 1024, 16, 64, 256
P = 128
SQ = 1024          # queries per core
SE = SQ + WIN      # ext tokens (halo + queries)
KB = H // P        # 8 contraction blocks
QBS = 256          # query block size in attention
NQB = SQ // QBS    # 4
JBN = 4            # key blocks of 128 per query block
LN_EPS = 1e-7
NCORES = 8

AF = mybir.ActivationFunctionType
ALU = mybir.AluOpType


def _bcast_ap(handle, n_part):
    """[D] DRAM vector -> [n_part, D] partition-broadcast AP (step 0)."""
    ap = handle[:]
    return bass.AP(tensor=ap.tensor, offset=ap.offset, ap=[[0, n_part]] + list(ap.ap))


def build_nc():
    nc = bacc.Bacc("TRN2", target_bir_lowering=False, debug=False)

    xq_h = nc.declare_dram_parameter("xq", [SQ, H], F32, isOutput=False)
    xT_h = nc.declare_dram_parameter("xT", [H, SE], BF16, isOutput=False)
    wqT_h = nc.declare_dram_parameter("wqT", [H, H], BF16, isOutput=False)
    wkT_h = nc.declare_dram_parameter("wkT", [H, H], BF16, isOutput=False)
    wvT_h = nc.declare_dram_parameter("wvT", [H, H], BF16, isOutput=False)
    woT_h = nc.declare_dram_parameter("woT", [H, H], BF16, isOutput=False)
    lnw_h = nc.declare_dram_parameter("lnw", [H], F32, isOutput=False)
    lnbbo_h = nc.declare_dram_parameter("lnbbo", [H], F32, isOutput=False)
    mask_h = nc.declare_dram_parameter("mask", [NQB, JBN, P, QBS], BF16,
                                       isOutput=False)
    out_h = nc.declare_dram_parameter("out", [SQ, H], F32, isOutput=True)

    with tile.TileContext(nc) as tc:
        _body(tc, nc, xq_h, xT_h, wqT_h, wkT_h, wvT_h, woT_h, lnw_h, lnbbo_h,
              mask_h, out_h)
    nc.compile()
    return nc


def _body(tc, nc, xq_h, xT_h, wqT_h, wkT_h, wvT_h, woT_h, lnw_h, lnbbo_h,
          mask_h, out_h):
    with (
        tc.tile_pool(name="const", bufs=1) as const,
        tc.tile_pool(name="big", bufs=1) as big,
        tc.tile_pool(name="wpool", bufs=10) as wpool,
        tc.tile_pool(name="work", bufs=3) as work,
        tc.tile_pool(name="lnpool", bufs=2) as lnpool,
        tc.tile_pool(name="respool", bufs=1) as respool,
    ):
        # ---- constants ----
        lnw_b = const.tile([P, H], F32)
        nc.sync.dma_start(lnw_b[:], _bcast_ap(lnw_h, P))
        lnbbo_b = const.tile([P, H], F32)
        nc.sync.dma_start(lnbbo_b[:], _bcast_ap(lnbbo_h, P))
        eps_t = const.tile([P, 1], F32)
        nc.vector.memset(eps_t[:], LN_EPS)
        mask_sb = const.tile([P, NQB, JBN, QBS], BF16)
        nc.sync.dma_start(mask_sb[:], mask_h[:].rearrange("qb jb p c -> p qb jb c"))

        # ---- x^T resident [128, kb, tok] ----
        xt_sb = big.tile([P, KB, SE], BF16)
        for kb in range(KB):
            nc.sync.dma_start(xt_sb[:, kb, :], xT_h[:][kb * P:(kb + 1) * P, :])

        qT_sb = big.tile([P, KB, SQ], BF16)    # q^T  [H, 1024]
        kT_sb = big.tile([P, KB, SE], BF16)    # k^T  [H, 1280]
        v_sb = big.tile([P, SE // P, NH, HD + 1], BF16)  # v natural + ones col
        nc.vector.memset(v_sb[:, :, :, HD:HD + 1], 1.0)
        ct_sb = big.tile([P, KB, SQ], BF16)    # normalized ctx^T [H, 1024]

        with tc.tile_pool(name="ppsum", bufs=4, space="PSUM") as ppsum:
            # ---- transposed projections: q^T, k^T ----
            for (w_h, dst, tok0, ntok) in ((wqT_h, qT_sb, WIN, SQ),
                                           (wkT_h, kT_sb, 0, SE)):
                wsl = [wpool.tile([P, H], BF16, tag="wslice", name=f"w_{kb}")
                       for kb in range(KB)]
                for kb in range(KB):
                    nc.sync.dma_start(wsl[kb][:], w_h[:][kb * P:(kb + 1) * P, :])
                chunks = [(i, min(512, ntok - i)) for i in range(0, ntok, 512)]
                for ob in range(KB):
                    for (i0, ilen) in chunks:
                        ps = ppsum.tile([P, 512], F32, tag="pj", name="ps_qk")
                        for kb in range(KB):
                            nc.tensor.matmul(
                                ps[:, :ilen],
                                wsl[kb][:, ob * P:(ob + 1) * P],
                                xt_sb[:, kb, tok0 + i0: tok0 + i0 + ilen],
                                start=(kb == 0), stop=(kb == KB - 1),
                            )
                        nc.scalar.copy(out=dst[:, ob, i0:i0 + ilen],
                                       in_=ps[:, :ilen])

            # ---- natural projection: v ----
            wsl = [wpool.tile([P, H], BF16, tag="wslice", name=f"wv_{kb}")
                   for kb in range(KB)]
            for kb in range(KB):
                nc.sync.dma_start(wsl[kb][:], wvT_h[:][kb * P:(kb + 1) * P, :])
            for tt in range(SE // P):
                for oh in range(2):
                    ps = ppsum.tile([P, 512], F32, tag="pj", name="ps_v")
                    for kb in range(KB):
                        nc.tensor.matmul(
                            ps[:],
                            xt_sb[:, kb, tt * P:(tt + 1) * P],
                            wsl[kb][:, oh * 512:(oh + 1) * 512],
                            start=(kb == 0), stop=(kb == KB - 1),
                        )
                    nc.scalar.copy(
                        out=v_sb[:, tt, oh * 8:(oh + 1) * 8, 0:HD],
                        in_=ps[:].rearrange("p (h d) -> p h d", d=HD),
                    )

            # ---- wo slices (prefetch; consumed at the end) ----
            wosl = [wpool.tile([P, H], BF16, tag="wslice", name=f"wo_{kb}")
                    for kb in range(KB)]
            for kb in range(KB):
                nc.sync.dma_start(wosl[kb][:], woT_h[:][kb * P:(kb + 1) * P, :])

            # ---- LayerNorm residual (DVE is idle during projections) ----
            res_sb = respool.tile([P, KB, H], F32)   # 8 query tiles x [128, 1024]
            for it in range(KB):
                x_t = lnpool.tile([P, H], F32, tag="x_t", name="x_t")
                nc.sync.dma_start(x_t[:], xq_h[:][it * P:(it + 1) * P, :])
                stats = lnpool.tile([P, 2, 6], F32, tag="stats", name="stats")
                for g in range(2):
                    nc.vector.bn_stats(out=stats[:, g, :],
                                       in_=x_t[:, g * 512:(g + 1) * 512])
                mv = lnpool.tile([P, 2], F32, tag="mv", name="mv")
                nc.vector.bn_aggr(out=mv[:], in_=stats[:])
                rstd = lnpool.tile([P, 1], F32, tag="rstd", name="rstd")
                nc.scalar.activation(out=rstd[:], in_=mv[:, 1:2], func=AF.Sqrt,
                                     bias=eps_t[:])
                nc.vector.reciprocal(out=rstd[:], in_=rstd[:])
                nc.vector.tensor_scalar(out=res_sb[:, it, :], in0=x_t[:],
                                        scalar1=mv[:, 0:1], scalar2=rstd[:],
                                        op0=ALU.subtract, op1=ALU.mult)
                nc.vector.tensor_mul(out=res_sb[:, it, :], in0=res_sb[:, it, :],
                                     in1=lnw_b[:])
                nc.vector.tensor_add(out=res_sb[:, it, :], in0=res_sb[:, it, :],
                                     in1=lnbbo_b[:])

        # ---- attention: scores^T -> exp -> mask -> ctx^T -> normalize ----
        with (
            tc.tile_pool(name="spsum", bufs=3, space="PSUM") as spsum,
            tc.tile_pool(name="cpsum", bufs=2, space="PSUM") as cpsum,
        ):
            for qb in range(NQB):
                for h in range(NH):
                    hb, ho = h // 2, (h % 2) * HD
                    ps_s = spsum.tile([P, JBN, QBS], F32, tag="sc", name="ps_s")
                    for jb in range(JBN):
                        j0 = qb * QBS + jb * P
                        nc.tensor.matmul(
                            ps_s[:, jb, :],
                            kT_sb[ho:ho + HD, hb, j0:j0 + P],
                            qT_sb[ho:ho + HD, hb, qb * QBS:(qb + 1) * QBS],
                            start=True, stop=True,
                        )
                    probs = work.tile([P, JBN, QBS], BF16, tag="probs",
                                      name="probs")
                    nc.scalar.activation(out=probs[:], in_=ps_s[:], func=AF.Exp)
                    nc.vector.tensor_mul(out=probs[:], in0=probs[:],
                                         in1=mask_sb[:, qb])
                    ps_c = cpsum.tile([HD + 1, QBS], F32, tag="cx", name="ps_c")
                    for jb in range(JBN):
                        nc.tensor.matmul(
                            ps_c[:],
                            v_sb[:, qb * 2 + jb, h, :],
                            probs[:, jb, :],
                            start=(jb == 0), stop=(jb == JBN - 1),
                        )
                    recip = work.tile([P, QBS], BF16, tag="recip", name="recip")
                    with nc.allow_low_precision(
                            reason="softmax denom recip in bf16: 0.4% rel on "
                                   "a 2e-2 budget"):
                        nc.vector.reciprocal(out=recip[0:1, :],
                                             in_=ps_c[HD:HD + 1, :])
                    rb = work.tile([HD, QBS], BF16, tag="rb", name="rb")
                    nc.gpsimd.partition_broadcast(rb[:], recip[0:1, :],
                                                  channels=HD)
                    nc.vector.tensor_mul(
                        out=ct_sb[ho:ho + HD, hb, qb * QBS:(qb + 1) * QBS],
                        in0=ps_c[0:HD, :], in1=rb[:],
                    )

        # ---- output projection + residual ----
        with tc.tile_pool(name="opsum", bufs=4, space="PSUM") as opsum:
            for it in range(KB):
                for oh in range(2):
                    ps_o = opsum.tile([P, 512], F32, tag="po", name="ps_o")
                    for hb in range(KB):
                        nc.tensor.matmul(
                            ps_o[:],
                            ct_sb[:, hb, it * P:(it + 1) * P],
                            wosl[hb][:, oh * 512:(oh + 1) * 512],
                            start=(hb == 0), stop=(hb == KB - 1),
                        )
                    o_t = work.tile([P, 512], F32, tag="o_t", name="o_t")
                    nc.vector.tensor_add(out=o_t[:], in0=ps_o[:],
                                         in1=res_sb[:, it, oh * 512:(oh + 1) * 512])
                    nc.sync.dma_start(
                        out_h[:][it * P:(it + 1) * P, oh * 512:(oh + 1) * 512],
                        o_t[:])


_CACHE = {}


def get_nc():
    if "nc" not in _CACHE:
        _CACHE["nc"] = build_nc()
    return _CACHE["nc"]


def make_in_maps(inputs):
    x = np.asarray(inputs["hidden_states"], dtype=np.float32)
    wq = np.asarray(inputs["wq"], dtype=np.float32)
    wk = np.asarray(inputs["wk"], dtype=np.float32)
    wv = np.asarray(inputs["wv"], dtype=np.float32)
    wo = np.asarray(inputs["wo"], dtype=np.float32)
    bo = np.asarray(inputs["bo"], dtype=np.float32)
    ln_w = np.asarray(inputs["ln_w"], dtype=np.float32)
    ln_b = np.asarray(inputs["ln_b"], dtype=np.float32)

    bf = ml_dtypes.bfloat16
    wqT = np.ascontiguousarray(wq.T).astype(bf)
    wkT = np.ascontiguousarray(wk.T).astype(bf)
    wvT = np.ascontiguousarray(wv.T).astype(bf)
    woT = np.ascontiguousarray(wo.T).astype(bf)
    lnbbo = (ln_b + bo).astype(np.float32)

    # band masks in extended coords: r = key row in 128-block, c = query col
    r = np.arange(P)[:, None]
    c = np.arange(QBS)[None, :]
    m = np.stack([(c < r), (c <= r + 127), (c >= r), (c >= r + 128)])
    mask_base = np.broadcast_to(m[None], (NQB, JBN, P, QBS)).astype(np.float32)

    in_maps = []
    for core in range(NCORES):
        b, hh = divmod(core, 2)
        start = hh * SQ
        xkv = np.zeros((SE, H), dtype=np.float32)
        xkv[WIN:] = x[b, start:start + SQ]
        if start > 0:
            xkv[:WIN] = x[b, start - WIN:start]
        mk = mask_base.copy()
        if start == 0:
            mk[0, 0] = 0.0
            mk[0, 1] = 0.0
        in_maps.append({
            "xq": np.ascontiguousarray(x[b, start:start + SQ]),
            "xT": np.ascontiguousarray(xkv.T).astype(bf),
            "wqT": wqT, "wkT": wkT, "wvT": wvT, "woT": woT,
            "lnw": ln_w, "lnbbo": lnbbo,
            "mask": mk.astype(bf),
        })
    return in_maps


def kernel(**inputs):
    from concourse.bass_utils import run_bass_kernel_spmd
    nc = get_nc()
    in_maps = make_in_maps(inputs)
    res = run_bass_kernel_spmd(nc, in_maps, core_ids=list(range(NCORES)))
    out = np.empty((B, S, H), dtype=np.float32)
    for core in range(NCORES):
        b, hh = divmod(core, 2)
        out[b, hh * SQ:(hh + 1) * SQ, :] = res.results[core]["out"]
    return out


# revision 21
# speedup vs baseline: 1.0879x; 1.0879x over previous
"""Trainium2 Bass kernel for AnyGPT local-attention block (8 NeuronCores).

Sharding: (batch, seq-half) -> 8 shards of 1024 query tokens each; every core
gets a 256-token k/v halo (zero-padded at sequence start), so no collectives
are needed and the host gather is a pure concatenation.

Per-core pipeline (all matmuls in bf16, LayerNorm/softmax math in fp32):
  qT/kT = W^T-major projections ([H, tok] layout), v natural ([tok, H]) with a
  built-in ones column for softmax denominators; banded scores computed
  TRANSPOSED ([key, query] layout) so exp'd probs feed the ctx matmul directly
  and ctx comes out pre-transposed for the output projection; softmax is
  unnormalized (no max subtraction; scores are O(30)) with the denominator
  recovered from the ones row and divided into ctx via a rank-1 broadcast.
"""

import numpy as np
import ml_dtypes

import concourse.bass as bass
import concourse.mybir as mybir
import concourse.tile as tile
from concourse import bacc

F32 = mybir.dt.float32
BF16 = mybir.dt.bfloat16

B, S, H, NH, HD, WIN = 4, 2048, 1024, 16, 64, 256
P = 128
SQ = 1024          # queries per core
SE = SQ + WIN      # ext tokens (halo + queries)
KB = H // P        # 8 contraction blocks
QBS = 256          # query block size in attention
NQB = SQ // QBS    # 4
JBN = 4            # key blocks of 128 per query block
LN_EPS = 1e-7
NCORES = 8

AF = mybir.ActivationFunctionType
ALU = mybir.AluOpType


def _bcast_ap(handle, n_part):
    """[D] DRAM vector -> [n_part, D] partition-broadcast AP (step 0)."""
    ap = handle[:]
    return bass.AP(tensor=ap.tensor, offset=ap.offset, ap=[[0, n_part]] + list(ap.ap))


def build_nc():
    nc = bacc.Bacc("TRN2", target_bir_lowering=False, debug=False)

    xq_h = nc.declare_dram_parameter("xq", [SQ, H], F32, isOutput=False)
    xT_h = nc.declare_dram_parameter("xT", [H, SE], BF16, isOutput=False)
    wqT_h = nc.declare_dram_parameter("wqT", [H, H], BF16, isOutput=False)
    wkT_h = nc.declare_dram_parameter("wkT", [H, H], BF16, isOutput=False)
    wvT_h = nc.declare_dram_parameter("wvT", [H, H], BF16, isOutput=False)
    woT_h = nc.declare_dram_parameter("woT", [H, H], BF16, isOutput=False)
    lnw_h = nc.declare_dram_parameter("lnw", [H], F32, isOutput=False)
    lnbbo_h = nc.declare_dram_parameter("lnbbo", [H], F32, isOutput=False)
    mask_h = nc.declare_dram_parameter("mask", [NQB, JBN, P, QBS], BF16,
                                       isOutput=False)
    sel_h = nc.declare_dram_parameter("sel", [NH, KB, P], BF16, isOutput=False)
    out_h = nc.declare_dram_parameter("out", [SQ, H], F32, isOutput=True)

    with tile.TileContext(nc) as tc:
        _body(tc, nc, xq_h, xT_h, wqT_h, wkT_h, wvT_h, woT_h, lnw_h, lnbbo_h,
              mask_h, sel_h, out_h)
    nc.compile()
    return nc


def _body(tc, nc, xq_h, xT_h, wqT_h, wkT_h, wvT_h, woT_h, lnw_h, lnbbo_h,
          mask_h, sel_h, out_h):
    with (
        tc.tile_pool(name="const", bufs=1) as const,
        tc.tile_pool(name="big", bufs=1) as big,
        tc.tile_pool(name="wpool", bufs=12) as wpool,
        tc.tile_pool(name="work", bufs=3) as work,
        tc.tile_pool(name="lnpool", bufs=2) as lnpool,
        tc.tile_pool(name="respool", bufs=1) as respool,
    ):
        # ---- constants ----
        lnw_b = const.tile([P, H], F32)
        nc.sync.dma_start(lnw_b[:], _bcast_ap(lnw_h, P))
        lnbbo_b = const.tile([P, H], F32)
        nc.sync.dma_start(lnbbo_b[:], _bcast_ap(lnbbo_h, P))
        eps_t = const.tile([P, 1], F32)
        nc.vector.memset(eps_t[:], LN_EPS)
        mask_sb = const.tile([P, NQB, JBN, QBS], BF16)
        nc.sync.dma_start(mask_sb[:], mask_h[:].rearrange("qb jb p c -> p qb jb c"))
        sel_sb = const.tile([NH, KB, P], BF16)
        nc.sync.dma_start(sel_sb[:], sel_h[:][:, :, :])

        # ---- x^T resident [128, kb, tok] ----
        xt_sb = big.tile([P, KB, SE], BF16)
        for kb in range(KB):
            nc.sync.dma_start(xt_sb[:, kb, :], xT_h[:][kb * P:(kb + 1) * P, :])

        qT_sb = big.tile([P, KB, SQ], BF16)    # q^T  [H, 1024]
        kT_sb = big.tile([P, KB, SE], BF16)    # k^T  [H, 1280]
        v_sb = big.tile([P, SE // P, NH, HD + 1], BF16)  # v natural + ones col
        nc.vector.memset(v_sb[:, :, :, HD:HD + 1], 1.0)
        ct_sb = big.tile([P, KB, SQ], BF16)    # UNnormalized ctx^T [H, 1024]
        den_sb = big.tile([NH, SQ], F32)       # softmax denominators [head, i]
        recip_sb = big.tile([NH, SQ], BF16)    # 1/den, bulk-reciprocated

        with tc.tile_pool(name="ppsum", bufs=4, space="PSUM") as ppsum:
            # ---- transposed projections: q^T, k^T ----
            for (w_h, dst, tok0, ntok) in ((wqT_h, qT_sb, WIN, SQ),
                                           (wkT_h, kT_sb, 0, SE)):
                wsl = [wpool.tile([P, H], BF16, tag="wslice", name=f"w_{kb}")
                       for kb in range(KB)]
                for kb in range(KB):
                    nc.sync.dma_start(wsl[kb][:], w_h[:][kb * P:(kb + 1) * P, :])
                chunks = [(i, min(512, ntok - i)) for i in range(0, ntok, 512)]
                for ob in range(KB):
                    for (i0, ilen) in chunks:
                        ps = ppsum.tile([P, 512], F32, tag="pj", name="ps_qk")
                        for kb in range(KB):
                            nc.tensor.matmul(
                                ps[:, :ilen],
                                wsl[kb][:, ob * P:(ob + 1) * P],
                                xt_sb[:, kb, tok0 + i0: tok0 + i0 + ilen],
                                start=(kb == 0), stop=(kb == KB - 1),
                            )
                        nc.scalar.copy(out=dst[:, ob, i0:i0 + ilen],
                                       in_=ps[:, :ilen])

            # ---- natural projection: v ----
            wsl = [wpool.tile([P, H], BF16, tag="wslice", name=f"wv_{kb}")
                   for kb in range(KB)]
            for kb in range(KB):
                nc.sync.dma_start(wsl[kb][:], wvT_h[:][kb * P:(kb + 1) * P, :])
            for tt in range(SE // P):
                for oh in range(2):
                    ps = ppsum.tile([P, 512], F32, tag="pj", name="ps_v")
                    for kb in range(KB):
                        nc.tensor.matmul(
                            ps[:],
                            xt_sb[:, kb, tt * P:(tt + 1) * P],
                            wsl[kb][:, oh * 512:(oh + 1) * 512],
                            start=(kb == 0), stop=(kb == KB - 1),
                        )
                    nc.scalar.copy(
                        out=v_sb[:, tt, oh * 8:(oh + 1) * 8, 0:HD],
                        in_=ps[:].rearrange("p (h d) -> p h d", d=HD),
                    )

            # ---- wo slices (prefetch; consumed at the end) ----
            wosl = [wpool.tile([P, H], BF16, tag="wslice", name=f"wo_{kb}")
                    for kb in range(KB)]
            for kb in range(KB):
                nc.sync.dma_start(wosl[kb][:], woT_h[:][kb * P:(kb + 1) * P, :])

            # ---- LayerNorm residual (DVE is idle during projections) ----
            res_sb = respool.tile([P, KB, H], F32)   # 8 query tiles x [128, 1024]
            for it in range(KB):
                x_t = lnpool.tile([P, H], F32, tag="x_t", name="x_t")
                nc.sync.dma_start(x_t[:], xq_h[:][it * P:(it + 1) * P, :])
                stats = lnpool.tile([P, 2, 6], F32, tag="stats", name="stats")
                for g in range(2):
                    nc.vector.bn_stats(out=stats[:, g, :],
                                       in_=x_t[:, g * 512:(g + 1) * 512])
                mv = lnpool.tile([P, 2], F32, tag="mv", name="mv")
                nc.vector.bn_aggr(out=mv[:], in_=stats[:])
                std = lnpool.tile([P, 1], F32, tag="std", name="std")
                nc.scalar.activation(out=std[:], in_=mv[:, 1:2], func=AF.Sqrt,
                                     bias=eps_t[:])
                rstd = lnpool.tile([P, 1], F32, tag="rstd", name="rstd")
                nc.vector.reciprocal_approx_fast(out=rstd[:], in_=std[:])
                nc.vector.tensor_scalar(out=res_sb[:, it, :], in0=x_t[:],
                                        scalar1=mv[:, 0:1], scalar2=rstd[:],
                                        op0=ALU.subtract, op1=ALU.mult)
                nc.vector.tensor_mul(out=res_sb[:, it, :], in0=res_sb[:, it, :],
                                     in1=lnw_b[:])
                nc.vector.tensor_add(out=res_sb[:, it, :], in0=res_sb[:, it, :],
                                     in1=lnbbo_b[:])

        # ---- attention: scores^T -> exp -> mask -> ctx^T -> normalize ----
        # Software-pipelined with a 2-iteration lookahead: the in-order PE
        # runs scores(i+2) while ACT/DVE turn scores(i) into masked probs, so
        # the ctx matmuls never stall the PE (and HAM stays warm).
        with (
            tc.tile_pool(name="spsum", bufs=3, space="PSUM") as spsum,
            tc.tile_pool(name="cpsum", bufs=2, space="PSUM") as cpsum,
        ):
            items = [(qb, h) for qb in range(NQB) for h in range(NH)]
            probs_of = {}

            def emit_scores(i):
                qb, h = items[i]
                hb, ho = h // 2, (h % 2) * HD
                ps_s = spsum.tile([P, JBN, QBS], F32, tag="sc", name="ps_s")
                for jb in range(JBN):
                    j0 = qb * QBS + jb * P
                    nc.tensor.matmul(
                        ps_s[:, jb, :],
                        kT_sb[ho:ho + HD, hb, j0:j0 + P],
                        qT_sb[ho:ho + HD, hb, qb * QBS:(qb + 1) * QBS],
                        start=True, stop=True,
                    )
                probs = work.tile([P, JBN, QBS], BF16, tag="probs",
                                  name="probs")
                nc.scalar.activation(out=probs[:], in_=ps_s[:], func=AF.Exp)
                nc.gpsimd.tensor_mul(out=probs[:], in0=probs[:],
                                     in1=mask_sb[:, qb])
                probs_of[i] = probs

            def emit_ctx(i):
                qb, h = items[i]
                hb, ho = h // 2, (h % 2) * HD
                probs = probs_of.pop(i)
                ps_c = cpsum.tile([HD + 1, QBS], F32, tag="cx", name="ps_c")
                for jb in range(JBN):
                    nc.tensor.matmul(
                        ps_c[:],
                        v_sb[:, qb * 2 + jb, h, :],
                        probs[:, jb, :],
                        start=(jb == 0), stop=(jb == JBN - 1),
                    )
                qs = slice(qb * QBS, (qb + 1) * QBS)
                nc.vector.tensor_copy(out=ct_sb[ho:ho + HD, hb, qs],
                                      in_=ps_c[0:HD, :])
                dstage = work.tile([1, QBS], F32, tag="dstage", name="dstage")
                nc.vector.tensor_copy(out=dstage[:], in_=ps_c[HD:HD + 1, :])
                nc.sync.dma_start(out=den_sb[h:h + 1, qs], in_=dstage[:])

            emit_scores(0)
            emit_scores(1)
            for i in range(len(items)):
                if i + 2 < len(items):
                    emit_scores(i + 2)
                emit_ctx(i)

            with nc.allow_low_precision(
                    reason="bulk softmax denom recip in bf16: 0.4% rel on a "
                           "2e-2 budget"):
                nc.vector.reciprocal(out=recip_sb[:], in_=den_sb[:])

        # ---- normalize ctx^T, then output projection + residual ----
        # R = selector-matmul broadcast of the per-head reciprocals into the
        # [128, 128] block layout of ct_sb (rows 0-63 <- even head, 64-127 <-
        # odd head), then ct_sb *= R in place.
        with (
            tc.tile_pool(name="opsum", bufs=4, space="PSUM") as opsum,
            tc.tile_pool(name="rpsum", bufs=2, space="PSUM") as rpsum,
        ):
            for it in range(KB):
                isl = slice(it * P, (it + 1) * P)
                for hb in range(KB):
                    ps_r = rpsum.tile([P, P], F32, tag="r", name="ps_r")
                    nc.tensor.matmul(ps_r[:], sel_sb[:, hb, :],
                                     recip_sb[:, isl], start=True, stop=True)
                    nc.vector.tensor_mul(out=ct_sb[:, hb, isl],
                                         in0=ct_sb[:, hb, isl], in1=ps_r[:])
                for oh in range(2):
                    ps_o = opsum.tile([P, 512], F32, tag="po", name="ps_o")
                    for hb in range(KB):
                        nc.tensor.matmul(
                            ps_o[:],
                            ct_sb[:, hb, it * P:(it + 1) * P],
                            wosl[hb][:, oh * 512:(oh + 1) * 512],
                            start=(hb == 0), stop=(hb == KB - 1),
                        )
                    o_t = work.tile([P, 512], F32, tag="o_t", name="o_t")
                    nc.vector.tensor_add(out=o_t[:], in0=ps_o[:],
                                         in1=res_sb[:, it, oh * 512:(oh + 1) * 512])
                    nc.sync.dma_start(
                        out_h[:][it * P:(it + 1) * P, oh * 512:(oh + 1) * 512],
                        o_t[:])


_CACHE = {}


def get_nc():
    if "nc" not in _CACHE:
        _CACHE["nc"] = build_nc()
    return _CACHE["nc"]


def make_in_maps(inputs):
    x = np.asarray(inputs["hidden_states"], dtype=np.float32)
    wq = np.asarray(inputs["wq"], dtype=np.float32)
    wk = np.asarray(inputs["wk"], dtype=np.float32)
    wv = np.asarray(inputs["wv"], dtype=np.float32)
    wo = np.asarray(inputs["wo"], dtype=np.float32)
    bo = np.asarray(inputs["bo"], dtype=np.float32)
    ln_w = np.asarray(inputs["ln_w"], dtype=np.float32)
    ln_b = np.asarray(inputs["ln_b"], dtype=np.float32)

    bf = ml_dtypes.bfloat16
    wqT = np.ascontiguousarray(wq.T).astype(bf)
    wkT = np.ascontiguousarray(wk.T).astype(bf)
    wvT = np.ascontiguousarray(wv.T).astype(bf)
    woT = np.ascontiguousarray(wo.T).astype(bf)
    lnbbo = (ln_b + bo).astype(np.float32)

    # band masks in extended coords: r = key row in 128-block, c = query col
    r = np.arange(P)[:, None]
    c = np.arange(QBS)[None, :]
    m = np.stack([(c < r), (c <= r + 127), (c >= r), (c >= r + 128)])
    mask_base = np.broadcast_to(m[None], (NQB, JBN, P, QBS)).astype(np.float32)

    # selector for the reciprocal broadcast: sel[p, hb, m] = 1 iff head p owns
    # row m of h-block hb in the ct layout (even head -> rows 0-63, odd -> 64+)
    sel = np.zeros((NH, KB, P), dtype=np.float32)
    for hb in range(KB):
        sel[2 * hb, hb, :HD] = 1.0
        sel[2 * hb + 1, hb, HD:] = 1.0
    sel = sel.astype(bf)

    in_maps = []
    for core in range(NCORES):
        b, hh = divmod(core, 2)
        start = hh * SQ
        xkv = np.zeros((SE, H), dtype=np.float32)
        xkv[WIN:] = x[b, start:start + SQ]
        if start > 0:
            xkv[:WIN] = x[b, start - WIN:start]
        mk = mask_base.copy()
        if start == 0:
            mk[0, 0] = 0.0
            mk[0, 1] = 0.0
        in_maps.append({
            "xq": np.ascontiguousarray(x[b, start:start + SQ]),
            "xT": np.ascontiguousarray(xkv.T).astype(bf),
            "wqT": wqT, "wkT": wkT, "wvT": wvT, "woT": woT,
            "lnw": ln_w, "lnbbo": lnbbo,
            "mask": mk.astype(bf),
            "sel": sel,
        })
    return in_maps


def kernel(**inputs):
    from concourse.bass_utils import run_bass_kernel_spmd
    nc = get_nc()
    in_maps = make_in_maps(inputs)
    res = run_bass_kernel_spmd(nc, in_maps, core_ids=list(range(NCORES)))
    out = np.empty((B, S, H), dtype=np.float32)
    for core in range(NCORES):
        b, hh = divmod(core, 2)
        out[b, hh * SQ:(hh + 1) * SQ, :] = res.results[core]["out"]
    return out


# revision 26
# speedup vs baseline: 1.1875x; 1.0915x over previous
"""Trainium2 Bass kernel for AnyGPT local-attention block (8 NeuronCores).

Sharding: (batch, seq-half) -> 8 shards of 1024 query tokens each; every core
gets a 256-token k/v halo (zero-padded at sequence start), so no collectives
are needed and the host gather is a pure concatenation.

Per-core pipeline (all matmuls in bf16, LayerNorm/softmax math in fp32):
  qT/kT = W^T-major projections ([H, tok] layout), v natural ([tok, H]) with a
  built-in ones column for softmax denominators; banded scores computed
  TRANSPOSED ([key, query] layout) so exp'd probs feed the ctx matmul directly
  and ctx comes out pre-transposed for the output projection; softmax is
  unnormalized (no max subtraction; scores are O(30)) with the denominator
  recovered from the ones row and divided into ctx via a rank-1 broadcast.
"""

import numpy as np
import ml_dtypes

import concourse.bass as bass
import concourse.mybir as mybir
import concourse.tile as tile
from concourse import bacc

F32 = mybir.dt.float32
BF16 = mybir.dt.bfloat16

B, S, H, NH, HD, WIN = 4, 2048, 1024, 16, 64, 256
P = 128
SQ = 1024          # queries per core
SE = SQ + WIN      # ext tokens (halo + queries)
KB = H // P        # 8 contraction blocks
QBS = 256          # query block size in attention
NQB = SQ // QBS    # 4
JBN = 4            # key blocks of 128 per query block
LN_EPS = 1e-7
NCORES = 8

AF = mybir.ActivationFunctionType
ALU = mybir.AluOpType


def _bcast_ap(handle, n_part):
    """[D] DRAM vector -> [n_part, D] partition-broadcast AP (step 0)."""
    ap = handle[:]
    return bass.AP(tensor=ap.tensor, offset=ap.offset, ap=[[0, n_part]] + list(ap.ap))


def build_nc():
    nc = bacc.Bacc("TRN2", target_bir_lowering=False, debug=False)

    xq_h = nc.declare_dram_parameter("xq", [SQ, H], F32, isOutput=False)
    xT_h = nc.declare_dram_parameter("xT", [H, SE], BF16, isOutput=False)
    wqT_h = nc.declare_dram_parameter("wqT", [H, H], BF16, isOutput=False)
    wkT_h = nc.declare_dram_parameter("wkT", [H, H], BF16, isOutput=False)
    wvT_h = nc.declare_dram_parameter("wvT", [H, H], BF16, isOutput=False)
    woT_h = nc.declare_dram_parameter("woT", [H, H], BF16, isOutput=False)
    lnw_h = nc.declare_dram_parameter("lnw", [H], F32, isOutput=False)
    lnbbo_h = nc.declare_dram_parameter("lnbbo", [H], F32, isOutput=False)
    mask_h = nc.declare_dram_parameter("mask", [NQB, JBN, P, QBS], BF16,
                                       isOutput=False)
    sel_h = nc.declare_dram_parameter("sel", [NH, KB, P], BF16, isOutput=False)
    out_h = nc.declare_dram_parameter("out", [SQ, H], F32, isOutput=True)

    with tile.TileContext(nc) as tc:
        _body(tc, nc, xq_h, xT_h, wqT_h, wkT_h, wvT_h, woT_h, lnw_h, lnbbo_h,
              mask_h, sel_h, out_h)
    nc.compile()
    return nc


def _body(tc, nc, xq_h, xT_h, wqT_h, wkT_h, wvT_h, woT_h, lnw_h, lnbbo_h,
          mask_h, sel_h, out_h):
    with (
        tc.tile_pool(name="const", bufs=1) as const,
        tc.tile_pool(name="big", bufs=1) as big,
        tc.tile_pool(name="wpool", bufs=12) as wpool,
        tc.tile_pool(name="work", bufs=3) as work,
        tc.tile_pool(name="lnpool", bufs=2) as lnpool,
        tc.tile_pool(name="respool", bufs=1) as respool,
    ):
        # ---- constants ----
        lnw_b = const.tile([P, H], F32)
        nc.sync.dma_start(lnw_b[:], _bcast_ap(lnw_h, P))
        lnbbo_b = const.tile([P, H], F32)
        nc.sync.dma_start(lnbbo_b[:], _bcast_ap(lnbbo_h, P))
        eps_t = const.tile([P, 1], F32)
        nc.vector.memset(eps_t[:], LN_EPS)
        mask_sb = const.tile([P, NQB, JBN, QBS], BF16)
        nc.sync.dma_start(mask_sb[:], mask_h[:].rearrange("qb jb p c -> p qb jb c"))
        sel_sb = const.tile([NH, KB, P], BF16)
        nc.sync.dma_start(sel_sb[:], sel_h[:][:, :, :])

        # ---- x^T resident [128, kb, tok]; interleave with the first weight
        # loads and split halves so the first matmul's operands land early ----
        xt_sb = big.tile([P, KB, SE], BF16)
        wq_sl = [wpool.tile([P, H], BF16, tag="wslice", name=f"wq_{kb}")
                 for kb in range(KB)]
        for kb in range(KB):
            nc.sync.dma_start(wq_sl[kb][:], wqT_h[:][kb * P:(kb + 1) * P, :])
            half = SE // 2
            nc.sync.dma_start(xt_sb[:, kb, :half],
                              xT_h[:][kb * P:(kb + 1) * P, :half])
            nc.sync.dma_start(xt_sb[:, kb, half:],
                              xT_h[:][kb * P:(kb + 1) * P, half:])

        qT_sb = big.tile([P, KB, SQ], BF16)    # q^T  [H, 1024]
        kT_sb = big.tile([P, KB, SE], BF16)    # k^T  [H, 1280]
        v_sb = big.tile([P, SE // P, NH, HD + 1], BF16)  # v natural + ones col
        nc.vector.memset(v_sb[:, :, :, HD:HD + 1], 1.0)
        ct_sb = big.tile([P, KB, SQ], BF16)    # UNnormalized ctx^T [H, 1024]
        den_sb = big.tile([NH, SQ], F32)       # softmax denominators [head, i]
        recip_sb = big.tile([NH, SQ], BF16)    # 1/den, bulk-reciprocated

        with tc.tile_pool(name="ppsum", bufs=4, space="PSUM") as ppsum:
            # ---- transposed projections: q^T, k^T ----
            for (w_h, dst, tok0, ntok, wsl) in ((wqT_h, qT_sb, WIN, SQ, wq_sl),
                                                (wkT_h, kT_sb, 0, SE, None)):
                if wsl is None:
                    wsl = [wpool.tile([P, H], BF16, tag="wslice",
                                      name=f"w_{kb}") for kb in range(KB)]
                    for kb in range(KB):
                        nc.sync.dma_start(wsl[kb][:],
                                          w_h[:][kb * P:(kb + 1) * P, :])
                chunks = [(i, min(512, ntok - i)) for i in range(0, ntok, 512)]
                for ob in range(KB):
                    for (i0, ilen) in chunks:
                        ps = ppsum.tile([P, 512], F32, tag="pj", name="ps_qk")
                        for kb in range(KB):
                            nc.tensor.matmul(
                                ps[:, :ilen],
                                wsl[kb][:, ob * P:(ob + 1) * P],
                                xt_sb[:, kb, tok0 + i0: tok0 + i0 + ilen],
                                start=(kb == 0), stop=(kb == KB - 1),
                            )
                        nc.scalar.copy(out=dst[:, ob, i0:i0 + ilen],
                                       in_=ps[:, :ilen])

            # ---- natural projection: v ----
            wsl = [wpool.tile([P, H], BF16, tag="wslice", name=f"wv_{kb}")
                   for kb in range(KB)]
            for kb in range(KB):
                nc.sync.dma_start(wsl[kb][:], wvT_h[:][kb * P:(kb + 1) * P, :])
            for tt in range(SE // P):
                for oh in range(2):
                    ps = ppsum.tile([P, 512], F32, tag="pj", name="ps_v")
                    for kb in range(KB):
                        nc.tensor.matmul(
                            ps[:],
                            xt_sb[:, kb, tt * P:(tt + 1) * P],
                            wsl[kb][:, oh * 512:(oh + 1) * 512],
                            start=(kb == 0), stop=(kb == KB - 1),
                        )
                    nc.scalar.copy(
                        out=v_sb[:, tt, oh * 8:(oh + 1) * 8, 0:HD],
                        in_=ps[:].rearrange("p (h d) -> p h d", d=HD),
                    )

            # ---- wo slices (prefetch; consumed at the end) ----
            wosl = [wpool.tile([P, H], BF16, tag="wslice", name=f"wo_{kb}")
                    for kb in range(KB)]
            for kb in range(KB):
                nc.sync.dma_start(wosl[kb][:], woT_h[:][kb * P:(kb + 1) * P, :])

            # ---- LayerNorm residual (DVE is idle during projections) ----
            res_sb = respool.tile([P, KB, H], F32)   # 8 query tiles x [128, 1024]
            for it in range(KB):
                x_t = lnpool.tile([P, H], F32, tag="x_t", name="x_t")
                nc.sync.dma_start(x_t[:], xq_h[:][it * P:(it + 1) * P, :])
                stats = lnpool.tile([P, 2, 6], F32, tag="stats", name="stats")
                for g in range(2):
                    nc.vector.bn_stats(out=stats[:, g, :],
                                       in_=x_t[:, g * 512:(g + 1) * 512])
                mv = lnpool.tile([P, 2], F32, tag="mv", name="mv")
                nc.vector.bn_aggr(out=mv[:], in_=stats[:])
                std = lnpool.tile([P, 1], F32, tag="std", name="std")
                nc.scalar.activation(out=std[:], in_=mv[:, 1:2], func=AF.Sqrt,
                                     bias=eps_t[:])
                rstd = lnpool.tile([P, 1], F32, tag="rstd", name="rstd")
                nc.vector.reciprocal_approx_fast(out=rstd[:], in_=std[:])
                nc.vector.tensor_scalar(out=res_sb[:, it, :], in0=x_t[:],
                                        scalar1=mv[:, 0:1], scalar2=rstd[:],
                                        op0=ALU.subtract, op1=ALU.mult)
                nc.vector.tensor_mul(out=res_sb[:, it, :], in0=res_sb[:, it, :],
                                     in1=lnw_b[:])
                nc.vector.tensor_add(out=res_sb[:, it, :], in0=res_sb[:, it, :],
                                     in1=lnbbo_b[:])

        # ---- attention: scores^T -> exp -> mask -> ctx^T -> normalize ----
        # Software-pipelined with a 2-iteration lookahead: the in-order PE
        # runs scores(i+2) while ACT/DVE turn scores(i) into masked probs, so
        # the ctx matmuls never stall the PE (and HAM stays warm).
        with (
            tc.tile_pool(name="spsum", bufs=3, space="PSUM") as spsum,
            tc.tile_pool(name="cpsum", bufs=2, space="PSUM") as cpsum,
        ):
            items = [(qb, h) for qb in range(NQB) for h in range(NH)]
            probs_of = {}

            def emit_scores(i):
                qb, h = items[i]
                hb, ho = h // 2, (h % 2) * HD
                ps_s = spsum.tile([P, JBN, QBS], F32, tag="sc", name="ps_s")
                for jb in range(JBN):
                    j0 = qb * QBS + jb * P
                    nc.tensor.matmul(
                        ps_s[:, jb, :],
                        kT_sb[ho:ho + HD, hb, j0:j0 + P],
                        qT_sb[ho:ho + HD, hb, qb * QBS:(qb + 1) * QBS],
                        start=True, stop=True,
                    )
                probs = work.tile([P, JBN, QBS], BF16, tag="probs",
                                  name="probs")
                nc.scalar.activation(out=probs[:], in_=ps_s[:], func=AF.Exp)
                # split the band-mask multiply across DVE and the otherwise
                # idle Pool engine so neither becomes the phase bottleneck
                nc.vector.tensor_mul(out=probs[:, 0:2, :], in0=probs[:, 0:2, :],
                                     in1=mask_sb[:, qb, 0:2, :])
                nc.gpsimd.tensor_mul(out=probs[:, 2:4, :], in0=probs[:, 2:4, :],
                                     in1=mask_sb[:, qb, 2:4, :])
                probs_of[i] = probs

            def emit_ctx(i):
                qb, h = items[i]
                hb, ho = h // 2, (h % 2) * HD
                probs = probs_of.pop(i)
                ps_c = cpsum.tile([HD + 1, QBS], F32, tag="cx", name="ps_c")
                for jb in range(JBN):
                    nc.tensor.matmul(
                        ps_c[:],
                        v_sb[:, qb * 2 + jb, h, :],
                        probs[:, jb, :],
                        start=(jb == 0), stop=(jb == JBN - 1),
                    )
                qs = slice(qb * QBS, (qb + 1) * QBS)
                nc.vector.tensor_copy(out=ct_sb[ho:ho + HD, hb, qs],
                                      in_=ps_c[0:HD, :])
                dstage = work.tile([1, QBS], F32, tag="dstage", name="dstage")
                nc.vector.tensor_copy(out=dstage[:], in_=ps_c[HD:HD + 1, :])
                nc.sync.dma_start(out=den_sb[h:h + 1, qs], in_=dstage[:])

            emit_scores(0)
            emit_scores(1)
            for i in range(len(items)):
                if i + 2 < len(items):
                    emit_scores(i + 2)
                emit_ctx(i)
                if (i + 1) % NH == 0:
                    # all heads of this query block done: reciprocate its
                    # denominator slice now so out-proj never waits on it
                    qb = items[i][0]
                    qs = slice(qb * QBS, (qb + 1) * QBS)
                    with nc.allow_low_precision(
                            reason="softmax denom recip in bf16: 0.4% rel "
                                   "on a 2e-2 budget"):
                        nc.vector.reciprocal(out=recip_sb[:, qs],
                                             in_=den_sb[:, qs])

        # ---- normalize ctx^T, then output projection + residual ----
        # R = selector-matmul broadcast of the per-head reciprocals into the
        # [128, 128] block layout of ct_sb (rows 0-63 <- even head, 64-127 <-
        # odd head), then ct_sb *= R in place.
        with (
            tc.tile_pool(name="opsum", bufs=4, space="PSUM") as opsum,
            tc.tile_pool(name="rpsum", bufs=4, space="PSUM") as rpsum,
        ):
            for it in range(KB):
                isl = slice(it * P, (it + 1) * P)
                for hb in range(KB):
                    ps_r = rpsum.tile([P, P], F32, tag="r", name="ps_r")
                    nc.tensor.matmul(ps_r[:], sel_sb[:, hb, :],
                                     recip_sb[:, isl], start=True, stop=True)
                    nc.vector.tensor_mul(out=ct_sb[:, hb, isl],
                                         in0=ct_sb[:, hb, isl], in1=ps_r[:])
            for it in range(KB):
                for oh in range(2):
                    ps_o = opsum.tile([P, 512], F32, tag="po", name="ps_o")
                    for hb in range(KB):
                        nc.tensor.matmul(
                            ps_o[:],
                            ct_sb[:, hb, it * P:(it + 1) * P],
                            wosl[hb][:, oh * 512:(oh + 1) * 512],
                            start=(hb == 0), stop=(hb == KB - 1),
                        )
                    o_t = work.tile([P, 512], F32, tag="o_t", name="o_t")
                    nc.vector.tensor_add(out=o_t[:], in0=ps_o[:],
                                         in1=res_sb[:, it, oh * 512:(oh + 1) * 512])
                    nc.sync.dma_start(
                        out_h[:][it * P:(it + 1) * P, oh * 512:(oh + 1) * 512],
                        o_t[:])


_CACHE = {}


def get_nc():
    if "nc" not in _CACHE:
        _CACHE["nc"] = build_nc()
    return _CACHE["nc"]


def make_in_maps(inputs):
    x = np.asarray(inputs["hidden_states"], dtype=np.float32)
    wq = np.asarray(inputs["wq"], dtype=np.float32)
    wk = np.asarray(inputs["wk"], dtype=np.float32)
    wv = np.asarray(inputs["wv"], dtype=np.float32)
    wo = np.asarray(inputs["wo"], dtype=np.float32)
    bo = np.asarray(inputs["bo"], dtype=np.float32)
    ln_w = np.asarray(inputs["ln_w"], dtype=np.float32)
    ln_b = np.asarray(inputs["ln_b"], dtype=np.float32)

    bf = ml_dtypes.bfloat16
    wqT = np.ascontiguousarray(wq.T).astype(bf)
    wkT = np.ascontiguousarray(wk.T).astype(bf)
    wvT = np.ascontiguousarray(wv.T).astype(bf)
    woT = np.ascontiguousarray(wo.T).astype(bf)
    lnbbo = (ln_b + bo).astype(np.float32)

    # band masks in extended coords: r = key row in 128-block, c = query col
    r = np.arange(P)[:, None]
    c = np.arange(QBS)[None, :]
    m = np.stack([(c < r), (c <= r + 127), (c >= r), (c >= r + 128)])
    mask_base = np.broadcast_to(m[None], (NQB, JBN, P, QBS)).astype(np.float32)

    # selector for the reciprocal broadcast: sel[p, hb, m] = 1 iff head p owns
    # row m of h-block hb in the ct layout (even head -> rows 0-63, odd -> 64+)
    sel = np.zeros((NH, KB, P), dtype=np.float32)
    for hb in range(KB):
        sel[2 * hb, hb, :HD] = 1.0
        sel[2 * hb + 1, hb, HD:] = 1.0
    sel = sel.astype(bf)

    in_maps = []
    for core in range(NCORES):
        b, hh = divmod(core, 2)
        start = hh * SQ
        xkv = np.zeros((SE, H), dtype=np.float32)
        xkv[WIN:] = x[b, start:start + SQ]
        if start > 0:
            xkv[:WIN] = x[b, start - WIN:start]
        mk = mask_base.copy()
        if start == 0:
            mk[0, 0] = 0.0
            mk[0, 1] = 0.0
        in_maps.append({
            "xq": np.ascontiguousarray(x[b, start:start + SQ]),
            "xT": np.ascontiguousarray(xkv.T).astype(bf),
            "wqT": wqT, "wkT": wkT, "wvT": wvT, "woT": woT,
            "lnw": ln_w, "lnbbo": lnbbo,
            "mask": mk.astype(bf),
            "sel": sel,
        })
    return in_maps


def kernel(**inputs):
    from concourse.bass_utils import run_bass_kernel_spmd
    nc = get_nc()
    in_maps = make_in_maps(inputs)
    res = run_bass_kernel_spmd(nc, in_maps, core_ids=list(range(NCORES)))
    out = np.empty((B, S, H), dtype=np.float32)
    for core in range(NCORES):
        b, hh = divmod(core, 2)
        out[b, hh * SQ:(hh + 1) * SQ, :] = res.results[core]["out"]
    return out


# revision 29
# speedup vs baseline: 1.2914x; 1.0876x over previous
"""Trainium2 Bass kernel for AnyGPT local-attention block (8 NeuronCores).

Sharding: (batch, seq-half) -> 8 shards of 1024 query tokens each; every core
gets a 256-token k/v halo (zero-padded at sequence start), so no collectives
are needed and the host gather is a pure concatenation.

Per-core pipeline (all matmuls in bf16, LayerNorm/softmax math in fp32):
  qT/kT = W^T-major projections ([H, tok] layout), v natural ([tok, H]) with a
  built-in ones column for softmax denominators; banded scores computed
  TRANSPOSED ([key, query] layout) so exp'd probs feed the ctx matmul directly
  and ctx comes out pre-transposed for the output projection; softmax is
  unnormalized (no max subtraction; scores are O(30)) with the denominator
  recovered from the ones row and divided into ctx via a rank-1 broadcast.
"""

import numpy as np
import ml_dtypes

import concourse.bass as bass
import concourse.mybir as mybir
import concourse.tile as tile
from concourse import bacc

F32 = mybir.dt.float32
BF16 = mybir.dt.bfloat16

B, S, H, NH, HD, WIN = 4, 2048, 1024, 16, 64, 256
P = 128
SQ = 1024          # queries per core
SE = SQ + WIN      # ext tokens (halo + queries)
KB = H // P        # 8 contraction blocks
QBS = 256          # query block size in attention
NQB = SQ // QBS    # 4
JBN = 4            # key blocks of 128 per query block
LN_EPS = 1e-7
NCORES = 8

AF = mybir.ActivationFunctionType
ALU = mybir.AluOpType


def _bcast_ap(handle, n_part):
    """[D] DRAM vector -> [n_part, D] partition-broadcast AP (step 0)."""
    ap = handle[:]
    return bass.AP(tensor=ap.tensor, offset=ap.offset, ap=[[0, n_part]] + list(ap.ap))


def build_nc():
    nc = bacc.Bacc("TRN2", target_bir_lowering=False, debug=False)

    xq_h = nc.declare_dram_parameter("xq", [SQ, H], F32, isOutput=False)
    xT_h = nc.declare_dram_parameter("xT", [H, SE], BF16, isOutput=False)
    wqT_h = nc.declare_dram_parameter("wqT", [H, H], BF16, isOutput=False)
    wkT_h = nc.declare_dram_parameter("wkT", [H, H], BF16, isOutput=False)
    wvT_h = nc.declare_dram_parameter("wvT", [H, H], BF16, isOutput=False)
    woT_h = nc.declare_dram_parameter("woT", [H, H], BF16, isOutput=False)
    lnw_h = nc.declare_dram_parameter("lnw", [H], F32, isOutput=False)
    lnbbo_h = nc.declare_dram_parameter("lnbbo", [H], F32, isOutput=False)
    mask_h = nc.declare_dram_parameter("mask", [NQB, JBN, P, QBS], BF16,
                                       isOutput=False)
    sel_h = nc.declare_dram_parameter("sel", [NH, KB, P], BF16, isOutput=False)
    out_h = nc.declare_dram_parameter("out", [SQ, H], F32, isOutput=True)

    with tile.TileContext(nc) as tc:
        _body(tc, nc, xq_h, xT_h, wqT_h, wkT_h, wvT_h, woT_h, lnw_h, lnbbo_h,
              mask_h, sel_h, out_h)
    nc.compile()
    return nc


def _body(tc, nc, xq_h, xT_h, wqT_h, wkT_h, wvT_h, woT_h, lnw_h, lnbbo_h,
          mask_h, sel_h, out_h):
    with (
        tc.tile_pool(name="const", bufs=1) as const,
        tc.tile_pool(name="big", bufs=1) as big,
        tc.tile_pool(name="wpool", bufs=12) as wpool,
        tc.tile_pool(name="work", bufs=3) as work,
        tc.tile_pool(name="lnpool", bufs=2) as lnpool,
        tc.tile_pool(name="respool", bufs=1) as respool,
    ):
        # ---- constants ----
        lnw_b = const.tile([P, H], F32)
        nc.sync.dma_start(lnw_b[:], _bcast_ap(lnw_h, P))
        lnbbo_b = const.tile([P, H], F32)
        nc.sync.dma_start(lnbbo_b[:], _bcast_ap(lnbbo_h, P))
        eps_t = const.tile([P, 1], F32)
        nc.vector.memset(eps_t[:], LN_EPS)
        mask_sb = const.tile([P, NQB, JBN, QBS], BF16)
        nc.sync.dma_start(mask_sb[:], mask_h[:].rearrange("qb jb p c -> p qb jb c"))
        sel_sb = const.tile([NH, KB, P], BF16)
        nc.sync.dma_start(sel_sb[:], sel_h[:][:, :, :])

        # ---- x^T resident [128, kb, tok]; interleave with the first weight
        # loads and split halves so the first matmul's operands land early ----
        xt_sb = big.tile([P, KB, SE], BF16)
        wq_sl = [wpool.tile([P, H], BF16, tag="wslice", name=f"wq_{kb}")
                 for kb in range(KB)]
        for kb in range(KB):
            nc.sync.dma_start(wq_sl[kb][:], wqT_h[:][kb * P:(kb + 1) * P, :])
            half = SE // 2
            nc.sync.dma_start(xt_sb[:, kb, :half],
                              xT_h[:][kb * P:(kb + 1) * P, :half])
            nc.sync.dma_start(xt_sb[:, kb, half:],
                              xT_h[:][kb * P:(kb + 1) * P, half:])

        qT_sb = big.tile([P, KB, SQ], BF16)    # q^T  [H, 1024]
        kT_sb = big.tile([P, KB, SE], BF16)    # k^T  [H, 1280]
        v_sb = big.tile([P, SE // P, NH, HD + 1], BF16)  # v natural + ones col
        nc.vector.memset(v_sb[:, :, :, HD:HD + 1], 1.0)
        ct_sb = big.tile([P, KB, SQ], BF16)    # UNnormalized ctx^T [H, 1024]
        den_sb = big.tile([NH, SQ], F32)       # softmax denominators [head, i]
        recip_sb = big.tile([NH, SQ], BF16)    # 1/den, bulk-reciprocated

        with tc.tile_pool(name="ppsum", bufs=4, space="PSUM") as ppsum:
            # ---- transposed projections: q^T, k^T ----
            for (w_h, dst, tok0, ntok, wsl) in ((wqT_h, qT_sb, WIN, SQ, wq_sl),
                                                (wkT_h, kT_sb, 0, SE, None)):
                if wsl is None:
                    wsl = [wpool.tile([P, H], BF16, tag="wslice",
                                      name=f"w_{kb}") for kb in range(KB)]
                    for kb in range(KB):
                        nc.sync.dma_start(wsl[kb][:],
                                          w_h[:][kb * P:(kb + 1) * P, :])
                chunks = [(i, min(512, ntok - i)) for i in range(0, ntok, 512)]
                for ob in range(KB):
                    for (i0, ilen) in chunks:
                        ps = ppsum.tile([P, 512], F32, tag="pj", name="ps_qk")
                        for kb in range(KB):
                            nc.tensor.matmul(
                                ps[:, :ilen],
                                wsl[kb][:, ob * P:(ob + 1) * P],
                                xt_sb[:, kb, tok0 + i0: tok0 + i0 + ilen],
                                start=(kb == 0), stop=(kb == KB - 1),
                            )
                        nc.scalar.copy(out=dst[:, ob, i0:i0 + ilen],
                                       in_=ps[:, :ilen])

            # ---- natural projection: v ----
            wsl = [wpool.tile([P, H], BF16, tag="wslice", name=f"wv_{kb}")
                   for kb in range(KB)]
            for kb in range(KB):
                nc.sync.dma_start(wsl[kb][:], wvT_h[:][kb * P:(kb + 1) * P, :])
            for tt in range(SE // P):
                for oh in range(2):
                    ps = ppsum.tile([P, 512], F32, tag="pj", name="ps_v")
                    for kb in range(KB):
                        nc.tensor.matmul(
                            ps[:],
                            xt_sb[:, kb, tt * P:(tt + 1) * P],
                            wsl[kb][:, oh * 512:(oh + 1) * 512],
                            start=(kb == 0), stop=(kb == KB - 1),
                        )
                    nc.scalar.copy(
                        out=v_sb[:, tt, oh * 8:(oh + 1) * 8, 0:HD],
                        in_=ps[:].rearrange("p (h d) -> p h d", d=HD),
                    )

            # ---- wo slices (prefetch; consumed at the end) ----
            wosl = [wpool.tile([P, H], BF16, tag="wslice", name=f"wo_{kb}")
                    for kb in range(KB)]
            for kb in range(KB):
                nc.sync.dma_start(wosl[kb][:], woT_h[:][kb * P:(kb + 1) * P, :])

            # ---- LayerNorm residual (DVE is idle during projections) ----
            res_sb = respool.tile([P, KB, H], F32)   # 8 query tiles x [128, 1024]
            for it in range(KB):
                x_t = lnpool.tile([P, H], F32, tag="x_t", name="x_t")
                nc.sync.dma_start(x_t[:], xq_h[:][it * P:(it + 1) * P, :])
                stats = lnpool.tile([P, 2, 6], F32, tag="stats", name="stats")
                for g in range(2):
                    nc.vector.bn_stats(out=stats[:, g, :],
                                       in_=x_t[:, g * 512:(g + 1) * 512])
                mv = lnpool.tile([P, 2], F32, tag="mv", name="mv")
                nc.vector.bn_aggr(out=mv[:], in_=stats[:])
                std = lnpool.tile([P, 1], F32, tag="std", name="std")
                nc.scalar.activation(out=std[:], in_=mv[:, 1:2], func=AF.Sqrt,
                                     bias=eps_t[:])
                rstd = lnpool.tile([P, 1], F32, tag="rstd", name="rstd")
                nc.vector.reciprocal_approx_fast(out=rstd[:], in_=std[:])
                nc.vector.tensor_scalar(out=res_sb[:, it, :], in0=x_t[:],
                                        scalar1=mv[:, 0:1], scalar2=rstd[:],
                                        op0=ALU.subtract, op1=ALU.mult)
                nc.vector.tensor_mul(out=res_sb[:, it, :], in0=res_sb[:, it, :],
                                     in1=lnw_b[:])
                nc.vector.tensor_add(out=res_sb[:, it, :], in0=res_sb[:, it, :],
                                     in1=lnbbo_b[:])

        # ---- attention: scores^T -> exp -> mask -> ctx^T -> normalize ----
        # Head-PAIR iterations: the even head's score matmuls contract on PE
        # rows 0-63, the odd head's on rows 64-127 (tile_position auto-derived
        # from the lhsT base partition), writing different PSUM banks, so the
        # hardware runs each jb's pair concurrently. Software-pipelined with a
        # 2-pair lookahead so the in-order PE never waits on exp/mask.
        with (
            tc.tile_pool(name="spsum", bufs=3, space="PSUM") as spsum,
            tc.tile_pool(name="cpsum", bufs=2, space="PSUM") as cpsum,
        ):
            pairs = [(qb, hb) for qb in range(NQB) for hb in range(NH // 2)]
            probs_of = {}

            def emit_scores(i):
                qb, hb = pairs[i]
                probs = work.tile([P, 2, JBN, QBS], BF16, tag="probs",
                                  name="probs")
                # two half-tiles of 2 jb x 2 parities (2 PSUM banks each) so
                # exp can drain each half while the next one is computed
                for half in range(2):
                    ps_s = spsum.tile([P, 2, 2, QBS], F32, tag="sc",
                                      name="ps_s")
                    for jbh in range(2):
                        jb = 2 * half + jbh
                        j0 = qb * QBS + jb * P
                        for par in range(2):
                            ho = par * HD
                            nc.tensor.matmul(
                                ps_s[:, par, jbh, :],
                                kT_sb[ho:ho + HD, hb, j0:j0 + P],
                                qT_sb[ho:ho + HD, hb,
                                      qb * QBS:(qb + 1) * QBS],
                                start=True, stop=True,
                            )
                    nc.scalar.activation(
                        out=probs[:, :, 2 * half:2 * half + 2, :],
                        in_=ps_s[:], func=AF.Exp)
                # band-mask multiply, broadcast over the parity dim; jb 0-2 on
                # DVE, jb 3 on the idle Pool engine (ctx consumes jb 3 last,
                # so the slower engine stays off the critical path)
                mq = mask_sb[:, qb]
                m_dve = bass.AP(tensor=mq.tensor, offset=mq.offset,
                                ap=[mq.ap[0], [0, 2], [mq.ap[1][0], 3],
                                    mq.ap[2]])
                nc.vector.tensor_mul(out=probs[:, :, 0:3, :],
                                     in0=probs[:, :, 0:3, :], in1=m_dve)
                mq3 = mask_sb[:, qb, 3, :]
                m_pool = bass.AP(tensor=mq3.tensor, offset=mq3.offset,
                                 ap=[mq3.ap[0], [0, 2], mq3.ap[1]])
                nc.gpsimd.tensor_mul(out=probs[:, :, 3, :],
                                     in0=probs[:, :, 3, :], in1=m_pool)
                probs_of[i] = probs

            def emit_ctx(i):
                qb, hb = pairs[i]
                probs = probs_of.pop(i)
                ps_c = cpsum.tile([HD + 1, 2, QBS], F32, tag="cx", name="ps_c")
                for par in range(2):
                    for jb in range(JBN):
                        nc.tensor.matmul(
                            ps_c[:, par, :],
                            v_sb[:, qb * 2 + jb, 2 * hb + par, :],
                            probs[:, par, jb, :],
                            start=(jb == 0), stop=(jb == JBN - 1),
                        )
                qs = slice(qb * QBS, (qb + 1) * QBS)
                nc.vector.tensor_copy(out=ct_sb[0:HD, hb, qs],
                                      in_=ps_c[0:HD, 0, :])
                nc.vector.tensor_copy(out=ct_sb[HD:P, hb, qs],
                                      in_=ps_c[0:HD, 1, :])
                dstage = work.tile([1, 2, QBS], F32, tag="dstage",
                                   name="dstage")
                nc.vector.tensor_copy(out=dstage[:], in_=ps_c[HD:HD + 1, :, :])
                for par in range(2):
                    nc.sync.dma_start(
                        out=den_sb[2 * hb + par:2 * hb + par + 1, qs],
                        in_=dstage[:, par, :])

            emit_scores(0)
            emit_scores(1)
            for i in range(len(pairs)):
                if i + 2 < len(pairs):
                    emit_scores(i + 2)
                emit_ctx(i)
                if (i + 1) % (NH // 2) == 0:
                    # all heads of this query block done: reciprocate its
                    # denominator slice now so out-proj never waits on it
                    qb = pairs[i][0]
                    qs = slice(qb * QBS, (qb + 1) * QBS)
                    with nc.allow_low_precision(
                            reason="softmax denom recip in bf16: 0.4% rel "
                                   "on a 2e-2 budget"):
                        nc.vector.reciprocal(out=recip_sb[:, qs],
                                             in_=den_sb[:, qs])

        # ---- normalize ctx^T, then output projection + residual ----
        # R = selector-matmul broadcast of the per-head reciprocals into the
        # [128, 128] block layout of ct_sb (rows 0-63 <- even head, 64-127 <-
        # odd head), then ct_sb *= R in place.
        with (
            tc.tile_pool(name="opsum", bufs=4, space="PSUM") as opsum,
            tc.tile_pool(name="rpsum", bufs=4, space="PSUM") as rpsum,
        ):
            for it in range(KB):
                isl = slice(it * P, (it + 1) * P)
                for hb in range(KB):
                    ps_r = rpsum.tile([P, P], F32, tag="r", name="ps_r")
                    nc.tensor.matmul(ps_r[:], sel_sb[:, hb, :],
                                     recip_sb[:, isl], start=True, stop=True)
                    nc.vector.tensor_mul(out=ct_sb[:, hb, isl],
                                         in0=ct_sb[:, hb, isl], in1=ps_r[:])
            for it in range(KB):
                for oh in range(2):
                    ps_o = opsum.tile([P, 512], F32, tag="po", name="ps_o")
                    for hb in range(KB):
                        nc.tensor.matmul(
                            ps_o[:],
                            ct_sb[:, hb, it * P:(it + 1) * P],
                            wosl[hb][:, oh * 512:(oh + 1) * 512],
                            start=(hb == 0), stop=(hb == KB - 1),
                        )
                    o_t = work.tile([P, 512], F32, tag="o_t", name="o_t")
                    nc.vector.tensor_add(out=o_t[:], in0=ps_o[:],
                                         in1=res_sb[:, it, oh * 512:(oh + 1) * 512])
                    nc.sync.dma_start(
                        out_h[:][it * P:(it + 1) * P, oh * 512:(oh + 1) * 512],
                        o_t[:])


_CACHE = {}


def get_nc():
    if "nc" not in _CACHE:
        _CACHE["nc"] = build_nc()
    return _CACHE["nc"]


def make_in_maps(inputs):
    x = np.asarray(inputs["hidden_states"], dtype=np.float32)
    wq = np.asarray(inputs["wq"], dtype=np.float32)
    wk = np.asarray(inputs["wk"], dtype=np.float32)
    wv = np.asarray(inputs["wv"], dtype=np.float32)
    wo = np.asarray(inputs["wo"], dtype=np.float32)
    bo = np.asarray(inputs["bo"], dtype=np.float32)
    ln_w = np.asarray(inputs["ln_w"], dtype=np.float32)
    ln_b = np.asarray(inputs["ln_b"], dtype=np.float32)

    bf = ml_dtypes.bfloat16
    wqT = np.ascontiguousarray(wq.T).astype(bf)
    wkT = np.ascontiguousarray(wk.T).astype(bf)
    wvT = np.ascontiguousarray(wv.T).astype(bf)
    woT = np.ascontiguousarray(wo.T).astype(bf)
    lnbbo = (ln_b + bo).astype(np.float32)

    # band masks in extended coords: r = key row in 128-block, c = query col
    r = np.arange(P)[:, None]
    c = np.arange(QBS)[None, :]
    m = np.stack([(c < r), (c <= r + 127), (c >= r), (c >= r + 128)])
    mask_base = np.broadcast_to(m[None], (NQB, JBN, P, QBS)).astype(np.float32)

    # selector for the reciprocal broadcast: sel[p, hb, m] = 1 iff head p owns
    # row m of h-block hb in the ct layout (even head -> rows 0-63, odd -> 64+)
    sel = np.zeros((NH, KB, P), dtype=np.float32)
    for hb in range(KB):
        sel[2 * hb, hb, :HD] = 1.0
        sel[2 * hb + 1, hb, HD:] = 1.0
    sel = sel.astype(bf)

    in_maps = []
    for core in range(NCORES):
        b, hh = divmod(core, 2)
        start = hh * SQ
        xkv = np.zeros((SE, H), dtype=np.float32)
        xkv[WIN:] = x[b, start:start + SQ]
        if start > 0:
            xkv[:WIN] = x[b, start - WIN:start]
        mk = mask_base.copy()
        if start == 0:
            mk[0, 0] = 0.0
            mk[0, 1] = 0.0
        in_maps.append({
            "xq": np.ascontiguousarray(x[b, start:start + SQ]),
            "xT": np.ascontiguousarray(xkv.T).astype(bf),
            "wqT": wqT, "wkT": wkT, "wvT": wvT, "woT": woT,
            "lnw": ln_w, "lnbbo": lnbbo,
            "mask": mk.astype(bf),
            "sel": sel,
        })
    return in_maps


def kernel(**inputs):
    from concourse.bass_utils import run_bass_kernel_spmd
    nc = get_nc()
    in_maps = make_in_maps(inputs)
    res = run_bass_kernel_spmd(nc, in_maps, core_ids=list(range(NCORES)))
    out = np.empty((B, S, H), dtype=np.float32)
    for core in range(NCORES):
        b, hh = divmod(core, 2)
        out[b, hh * SQ:(hh + 1) * SQ, :] = res.results[core]["out"]
    return out
